# revision 11
# baseline (speedup 1.0000x reference)
"""Causal single-head attention  B=4, T=4096, C=1024, D=64  on 8 TRN2 cores.

Sharding: 2 cores per batch; core parity p takes query rows {2i+p}.
Even/odd interleave balances causal work exactly across the pair.

v3 (bf16, schedule-optimized):
  - projections of block-pair bp are interleaved with attention tile bp, so
    ACT starts exp'ing ~3us in instead of after all projections; total ACT
    (~40us) hides under total PE (~51us).
  - score psums allocated as (128, 2, 512) chunk-PAIRS: one exp per
    off-diagonal pair (56 ACT instructions instead of 80).
  - kT stored per-parity at partitions 0:64 -> single qT tile serves both
    parities (no duplication copy).
  - everything else as the v1 baseline: bf16 matmuls, unstabilized exp
    (scale 0.125), diagonal-chunk 0/1 masks on DVE, out row 64 =
    denominator, host divides.
"""

import sys

sys.path.insert(0, "/opt/trn_rl_repo")

import numpy as np
import ml_dtypes

B, T, C, D = 4, 4096, 1024, 64
TQ = T // 2            # local queries per core
NT = 4                 # q tiles of 512
QF = 512               # q free-dim tile
KC = 128               # kv chunk
NCC = C // 128         # 8 contraction chunks
N_CORES = 8

_compiled = None


def _build_nc(loop_n=None):
    import contextlib
    import concourse.bass as bass
    import concourse.bacc as bacc
    import concourse.mybir as mybir
    from concourse.tile import TileContext
    from concourse.masks import make_identity

    f32 = mybir.dt.float32
    bf16 = mybir.dt.bfloat16

    nc = bacc.Bacc("TRN2", target_bir_lowering=False, debug=False)
    xt = nc.dram_tensor("xt", (C, T), bf16, kind="ExternalInput")
    wq = nc.dram_tensor("wq", (128, NCC * D), bf16, kind="ExternalInput")
    wkv = nc.dram_tensor("wkv", (128, NCC * 2 * D), bf16, kind="ExternalInput")
    masks = nc.dram_tensor("masks", (KC, 8 * QF), bf16, kind="ExternalInput")
    out = nc.dram_tensor("out", (D + 1, TQ), f32, kind="ExternalOutput")

    with TileContext(nc) as tc:
        with (
            tc.tile_pool(name="const", bufs=1) as constp,
            tc.tile_pool(name="xtp", bufs=66) as xtp,
            tc.tile_pool(name="ktp", bufs=8) as ktp,
            tc.tile_pool(name="vtp", bufs=3) as vtp,
            tc.tile_pool(name="qtp", bufs=4) as qtp,
            tc.tile_pool(name="vaug", bufs=33) as vaugp,
            tc.tile_pool(name="probs", bufs=6) as probsp,
            tc.tile_pool(name="osb", bufs=2) as osbp,
            tc.tile_pool(name="ps_kv", bufs=1, space="PSUM") as ps_kvp,
            tc.tile_pool(name="ps_scr", bufs=2, space="PSUM") as ps_scrp,
            tc.tile_pool(name="ps_s", bufs=2, space="PSUM") as ps_sp,
            tc.tile_pool(name="ps_o", bufs=1, space="PSUM") as ps_op,
        ):
            ident = constp.tile([64, 64], bf16, tag="ident")
            make_identity(nc, ident)

            wq_sb = constp.tile([128, NCC * D], bf16, tag="wq")
            wkv_sb = constp.tile([128, NCC * 2 * D], bf16, tag="wkv")
            mask_sb = constp.tile([128, 8 * QF], bf16, tag="masks")
            nc.sync.dma_start(out=wq_sb, in_=wq[:, :])
            nc.sync.dma_start(out=wkv_sb, in_=wkv[:, :])
            nc.sync.dma_start(out=mask_sb, in_=masks[:, :])

            loop_cm = (
                tc.For_i(0, loop_n, 1) if loop_n else contextlib.nullcontext()
            )
            with loop_cm:
              kts = {}     # block b (0-3 e, 4-7 o) -> (64, 512) bf16 kT
              qts = {}     # q tile t -> (64, 512) bf16
              vaug = {}    # kv chunk id (s*16 + 4*bp + j) -> (128, 65) bf16

              def load_block(b):
                  xts = []
                  for c in range(NCC):
                      xtile = xtp.tile([128, QF], bf16, tag="xt")
                      eng = nc.sync if c % 2 == 0 else nc.gpsimd
                      eng.dma_start(
                          out=xtile,
                          in_=xt[c * 128:(c + 1) * 128, b * QF:(b + 1) * QF],
                      )
                      xts.append(xtile)
                  return xts

              def proj_block(bp, s):
                  b = bp + 4 * s
                  xts = load_block(b)
                  ps_kv = ps_kvp.tile([128, QF], f32, tag="pskv")
                  for c in range(NCC):
                      nc.tensor.matmul(
                          ps_kv,
                          lhsT=wkv_sb[:, c * 2 * D:(c + 1) * 2 * D],
                          rhs=xts[c],
                          start=(c == 0),
                          stop=(c == NCC - 1),
                      )
                  kt_t = ktp.tile([64, QF], bf16, tag="kt")
                  nc.vector.tensor_copy(kt_t, ps_kv[0:64, :])
                  kts[b] = kt_t
                  vt_b = vtp.tile([64, QF], bf16, tag="vt")
                  nc.vector.tensor_copy(vt_b, ps_kv[64:128, :])
                  for j in range(4):
                      ps_v = ps_scrp.tile([128, 64], bf16, tag="scr")
                      nc.tensor.transpose(
                          ps_v, vt_b[:, j * 128:(j + 1) * 128], ident
                      )
                      va = vaugp.tile([128, D + 1], bf16, tag="vaug")
                      nc.vector.tensor_copy(va[:, 0:D], ps_v)
                      nc.gpsimd.memset(va[:, D:D + 1], 1.0)
                      vaug[s * 16 + 4 * bp + j] = va
                  if s == 0:
                      ps_q = ps_scrp.tile([64, QF], f32, tag="scr")
                      for c in range(NCC):
                          nc.tensor.matmul(
                              ps_q,
                              lhsT=wq_sb[:, c * D:(c + 1) * D],
                              rhs=xts[c],
                              start=(c == 0),
                              stop=(c == NCC - 1),
                          )
                      qt_t = qtp.tile([64, QF], bf16, tag="qt")
                      nc.vector.tensor_copy(qt_t, ps_q)
                      qts[bp] = qt_t

              def attn_tile(t):
                qt = qts[t]
                ps_o = ps_op.tile([D + 1, QF], f32, tag="pso")
                # pair = (kind, s, block bp, base chunk j0)
                pairs = []
                for bp in range(t):
                    for s in (0, 1):
                        for c in (0, 1):
                            pairs.append(("off", s, bp, 2 * c))
                for s in (0, 1):
                    for rr in (0, 2):
                        pairs.append(("diag", s, t, rr))
                np_ = len(pairs)
                pend = {}

                def emit_scores(i):
                    kind, s, bp, j0 = pairs[i]
                    kt_t = kts[bp + 4 * s]
                    psp = ps_sp.tile([128, 2, QF], f32, tag="pss")
                    for u in (0, 1):
                        j = j0 + u
                        c0 = 128 * j if kind == "diag" else 0
                        nc.tensor.matmul(
                            psp[:, u, c0:],
                            lhsT=kt_t[:, j * 128:(j + 1) * 128],
                            rhs=qt[:, c0:],
                            start=True,
                            stop=True,
                        )
                    pend[i] = psp

                def emit_rest(i, first, last):
                    kind, s, bp, j0 = pairs[i]
                    psp = pend.pop(i)
                    probs = probsp.tile([128, 2, QF], bf16, tag="p")
                    if kind == "off":
                        nc.scalar.activation(
                            probs, psp,
                            mybir.ActivationFunctionType.Exp, scale=0.125,
                        )
                    for u in (0, 1):
                        j = j0 + u
                        c0 = 128 * j if kind == "diag" else 0
                        if kind == "diag":
                            nc.scalar.activation(
                                probs[:, u, c0:], psp[:, u, c0:],
                                mybir.ActivationFunctionType.Exp, scale=0.125,
                            )
                            mi = j + (0 if s == 0 else 4)
                            nc.vector.tensor_mul(
                                probs[:, u, c0:],
                                probs[:, u, c0:],
                                mask_sb[:, mi * QF + c0:(mi + 1) * QF],
                            )
                        nc.tensor.matmul(
                            ps_o[:, c0:],
                            lhsT=vaug[s * 16 + 4 * bp + j],
                            rhs=probs[:, u, c0:],
                            start=(first and u == 0),
                            stop=(last and u == 1),
                            skip_group_check=True,
                        )

                for i in range(np_ + 1):
                    if i < np_:
                        emit_scores(i)
                    if i >= 1:
                        emit_rest(i - 1, first=(i - 1 == 0),
                                  last=(i - 1 == np_ - 1))
                o_sb = osbp.tile([D + 1, QF], f32, tag="osb")
                nc.vector.tensor_copy(o_sb, ps_o)
                nc.sync.dma_start(
                    out=out[:, t * QF:(t + 1) * QF], in_=o_sb
                )

              # interleave: projections of pair bp, then attention tile bp
              for bp in range(4):
                  proj_block(bp, 0)
                  proj_block(bp, 1)
                  attn_tile(bp)

    nc.compile()
    return nc


def _get_compiled():
    global _compiled
    if _compiled is None:
        _compiled = _build_nc()
    return _compiled


def _host_inputs(x, Wq, Wk, Wv):
    bf = ml_dtypes.bfloat16
    # pack (1024, d) weights chunk-wise along columns: (128, 8*d)
    wq = np.concatenate(
        [Wq[c * 128:(c + 1) * 128] for c in range(C // 128)], axis=1
    ).astype(bf)
    wkv_full = np.concatenate([Wk, Wv], axis=1)
    wkv = np.concatenate(
        [wkv_full[c * 128:(c + 1) * 128] for c in range(C // 128)], axis=1
    ).astype(bf)

    j = np.arange(KC)[:, None]   # kv row within chunk
    i = np.arange(QF)[None, :]   # q col within tile
    in_maps = []
    for core in range(N_CORES):
        b, p = core // 2, core % 2
        xs = x[b, p::2]          # (2048, 1024) same parity
        xo = x[b, 1 - p::2]
        xkvT = np.concatenate([xs, xo], axis=0).T
        xkvT = np.ascontiguousarray(xkvT, dtype=bf)
        ms = [(j <= i - 128 * r).astype(bf) for r in range(4)]
        mo = [(j <= i - 128 * r - (1 - p)).astype(bf) for r in range(4)]
        mask = np.concatenate(ms + mo, axis=1)   # (128, 8*512)
        in_maps.append({"xt": xkvT, "wq": wq, "wkv": wkv, "masks": mask})
    return in_maps


def kernel(x, Wq, Wk, Wv):
    from concourse.bass_utils import run_bass_kernel_spmd

    nc = _get_compiled()
    in_maps = _host_inputs(x, Wq, Wk, Wv)
    res = run_bass_kernel_spmd(nc, in_maps, core_ids=list(range(N_CORES)))

    out_full = np.empty((B, T, D), dtype=np.float32)
    for core in range(N_CORES):
        b, p = core // 2, core % 2
        acc = res.results[core]["out"]          # (65, 2048) f32
        out_full[b, p::2, :] = (acc[:D] / acc[D:D + 1]).T
    return out_full


# revision 13
# speedup vs baseline: 1.1103x; 1.1103x over previous
"""Causal single-head attention  B=4, T=4096, C=1024, D=64  on 8 TRN2 cores.

Sharding: 2 cores per batch; core parity p takes query rows {2i+p}.
Even/odd interleave balances causal work exactly across the pair.

v3 (bf16, schedule-optimized):
  - projections of block-pair bp are interleaved with attention tile bp, so
    ACT starts exp'ing ~3us in instead of after all projections; total ACT
    (~40us) hides under total PE (~51us).
  - score psums allocated as (128, 2, 512) chunk-PAIRS: one exp per
    off-diagonal pair (56 ACT instructions instead of 80).
  - kT stored per-parity at partitions 0:64 -> single qT tile serves both
    parities (no duplication copy).
  - everything else as the v1 baseline: bf16 matmuls, unstabilized exp
    (scale 0.125), diagonal-chunk 0/1 masks on DVE, out row 64 =
    denominator, host divides.
"""

import sys

sys.path.insert(0, "/opt/trn_rl_repo")

import numpy as np
import ml_dtypes

B, T, C, D = 4, 4096, 1024, 64
TQ = T // 2            # local queries per core
NT = 4                 # q tiles of 512
QF = 512               # q free-dim tile
KC = 128               # kv chunk
NCC = C // 128         # 8 contraction chunks
N_CORES = 8

_compiled = None


def _build_nc(loop_n=None):
    import contextlib
    import concourse.bass as bass
    import concourse.bacc as bacc
    import concourse.mybir as mybir
    from concourse.tile import TileContext
    from concourse.masks import make_identity

    f32 = mybir.dt.float32
    bf16 = mybir.dt.bfloat16

    nc = bacc.Bacc("TRN2", target_bir_lowering=False, debug=False)
    xt = nc.dram_tensor("xt", (C, T), bf16, kind="ExternalInput")
    wq = nc.dram_tensor("wq", (128, NCC * D), bf16, kind="ExternalInput")
    wkv = nc.dram_tensor("wkv", (128, NCC * 2 * D), bf16, kind="ExternalInput")
    masks = nc.dram_tensor("masks", (KC, 8 * QF), bf16, kind="ExternalInput")
    out = nc.dram_tensor("out", (D + 1, TQ), f32, kind="ExternalOutput")

    with TileContext(nc) as tc:
        with (
            tc.tile_pool(name="const", bufs=1) as constp,
            tc.tile_pool(name="xtp", bufs=66) as xtp,
            tc.tile_pool(name="ktp", bufs=8) as ktp,
            tc.tile_pool(name="vtp", bufs=3) as vtp,
            tc.tile_pool(name="qtp", bufs=4) as qtp,
            tc.tile_pool(name="vaug", bufs=33) as vaugp,
            tc.tile_pool(name="probs", bufs=6) as probsp,
            tc.tile_pool(name="osb", bufs=2) as osbp,
            tc.tile_pool(name="ps_kv", bufs=1, space="PSUM") as ps_kvp,
            tc.tile_pool(name="ps_scr", bufs=2, space="PSUM") as ps_scrp,
            tc.tile_pool(name="ps_s", bufs=2, space="PSUM") as ps_sp,
            tc.tile_pool(name="ps_o", bufs=1, space="PSUM") as ps_op,
        ):
            ident = constp.tile([64, 64], bf16, tag="ident")
            make_identity(nc, ident)

            wq_sb = constp.tile([128, NCC * D], bf16, tag="wq")
            wkv_sb = constp.tile([128, NCC * 2 * D], bf16, tag="wkv")
            mask_sb = constp.tile([128, 8 * QF], bf16, tag="masks")
            nc.sync.dma_start(out=wq_sb, in_=wq[:, :])
            nc.sync.dma_start(out=wkv_sb, in_=wkv[:, :])
            nc.sync.dma_start(out=mask_sb, in_=masks[:, :])

            loop_cm = (
                tc.For_i(0, loop_n, 1) if loop_n else contextlib.nullcontext()
            )
            with loop_cm:
              kts = {}     # block b (0-3 e, 4-7 o) -> (64, 512) bf16 kT
              qts = {}     # q tile t -> (64, 512) bf16
              vaug = {}    # kv chunk id (s*16 + 4*bp + j) -> (128, 65) bf16

              def load_block(b):
                  xts = []
                  for c in range(NCC):
                      xtile = xtp.tile([128, QF], bf16, tag="xt")
                      nc.sync.dma_start(
                          out=xtile,
                          in_=xt[c * 128:(c + 1) * 128, b * QF:(b + 1) * QF],
                      )
                      xts.append(xtile)
                  return xts

              def proj_block(bp, s):
                  b = bp + 4 * s
                  xts = load_block(b)
                  ps_kv = ps_kvp.tile([128, QF], f32, tag="pskv")
                  for c in range(NCC):
                      nc.tensor.matmul(
                          ps_kv,
                          lhsT=wkv_sb[:, c * 2 * D:(c + 1) * 2 * D],
                          rhs=xts[c],
                          start=(c == 0),
                          stop=(c == NCC - 1),
                      )
                  kt_t = ktp.tile([64, QF], bf16, tag="kt")
                  nc.vector.tensor_copy(kt_t, ps_kv[0:64, :])
                  kts[b] = kt_t
                  vt_b = vtp.tile([64, QF], bf16, tag="vt")
                  nc.vector.tensor_copy(vt_b, ps_kv[64:128, :])
                  for j in range(4):
                      ps_v = ps_scrp.tile([128, 64], bf16, tag="scr")
                      nc.tensor.transpose(
                          ps_v, vt_b[:, j * 128:(j + 1) * 128], ident
                      )
                      va = vaugp.tile([128, D + 1], bf16, tag="vaug")
                      nc.vector.tensor_copy(va[:, 0:D], ps_v)
                      nc.vector.memset(va[:, D:D + 1], 1.0)
                      vaug[s * 16 + 4 * bp + j] = va
                  if s == 0:
                      ps_q = ps_scrp.tile([64, QF], f32, tag="scr")
                      for c in range(NCC):
                          nc.tensor.matmul(
                              ps_q,
                              lhsT=wq_sb[:, c * D:(c + 1) * D],
                              rhs=xts[c],
                              start=(c == 0),
                              stop=(c == NCC - 1),
                          )
                      qt_t = qtp.tile([64, QF], bf16, tag="qt")
                      nc.vector.tensor_copy(qt_t, ps_q)
                      qts[bp] = qt_t

              def attn_tile(t):
                qt = qts[t]
                ps_o = ps_op.tile([D + 1, QF], f32, tag="pso")
                # pair = (kind, s, block bp, base chunk j0)
                pairs = []
                for bp in range(t):
                    for s in (0, 1):
                        for c in (0, 1):
                            pairs.append(("off", s, bp, 2 * c))
                for s in (0, 1):
                    for rr in (0, 2):
                        pairs.append(("diag", s, t, rr))
                np_ = len(pairs)
                pend = {}

                def emit_scores(i):
                    kind, s, bp, j0 = pairs[i]
                    kt_t = kts[bp + 4 * s]
                    psp = ps_sp.tile([128, 2, QF], f32, tag="pss")
                    for u in (0, 1):
                        j = j0 + u
                        c0 = 128 * j if kind == "diag" else 0
                        nc.tensor.matmul(
                            psp[:, u, c0:],
                            lhsT=kt_t[:, j * 128:(j + 1) * 128],
                            rhs=qt[:, c0:],
                            start=True,
                            stop=True,
                        )
                    pend[i] = psp

                def emit_rest(i, first, last):
                    kind, s, bp, j0 = pairs[i]
                    psp = pend.pop(i)
                    probs = probsp.tile([128, 2, QF], bf16, tag="p")
                    if kind == "off":
                        nc.scalar.activation(
                            probs, psp,
                            mybir.ActivationFunctionType.Exp, scale=0.125,
                        )
                    for u in (0, 1):
                        j = j0 + u
                        c0 = 128 * j if kind == "diag" else 0
                        if kind == "diag":
                            nc.scalar.activation(
                                probs[:, u, c0:], psp[:, u, c0:],
                                mybir.ActivationFunctionType.Exp, scale=0.125,
                            )
                            mi = j + (0 if s == 0 else 4)
                            nc.vector.tensor_mul(
                                probs[:, u, c0:],
                                probs[:, u, c0:],
                                mask_sb[:, mi * QF + c0:(mi + 1) * QF],
                            )
                        nc.tensor.matmul(
                            ps_o[:, c0:],
                            lhsT=vaug[s * 16 + 4 * bp + j],
                            rhs=probs[:, u, c0:],
                            start=(first and u == 0),
                            stop=(last and u == 1),
                            skip_group_check=True,
                        )

                for i in range(np_ + 1):
                    if i < np_:
                        emit_scores(i)
                    if i >= 1:
                        emit_rest(i - 1, first=(i - 1 == 0),
                                  last=(i - 1 == np_ - 1))
                o_sb = osbp.tile([D + 1, QF], f32, tag="osb")
                nc.vector.tensor_copy(o_sb, ps_o)
                nc.gpsimd.dma_start(
                    out=out[:, t * QF:(t + 1) * QF], in_=o_sb
                )

              # interleave: projections of pair bp, then attention tile bp
              for bp in range(4):
                  proj_block(bp, 0)
                  proj_block(bp, 1)
                  attn_tile(bp)

    nc.compile()
    return nc


def _get_compiled():
    global _compiled
    if _compiled is None:
        _compiled = _build_nc()
    return _compiled


def _host_inputs(x, Wq, Wk, Wv):
    bf = ml_dtypes.bfloat16
    # pack (1024, d) weights chunk-wise along columns: (128, 8*d)
    wq = np.concatenate(
        [Wq[c * 128:(c + 1) * 128] for c in range(C // 128)], axis=1
    ).astype(bf)
    wkv_full = np.concatenate([Wk, Wv], axis=1)
    wkv = np.concatenate(
        [wkv_full[c * 128:(c + 1) * 128] for c in range(C // 128)], axis=1
    ).astype(bf)

    j = np.arange(KC)[:, None]   # kv row within chunk
    i = np.arange(QF)[None, :]   # q col within tile
    in_maps = []
    for core in range(N_CORES):
        b, p = core // 2, core % 2
        xs = x[b, p::2]          # (2048, 1024) same parity
        xo = x[b, 1 - p::2]
        xkvT = np.concatenate([xs, xo], axis=0).T
        xkvT = np.ascontiguousarray(xkvT, dtype=bf)
        ms = [(j <= i - 128 * r).astype(bf) for r in range(4)]
        mo = [(j <= i - 128 * r - (1 - p)).astype(bf) for r in range(4)]
        mask = np.concatenate(ms + mo, axis=1)   # (128, 8*512)
        in_maps.append({"xt": xkvT, "wq": wq, "wkv": wkv, "masks": mask})
    return in_maps


def kernel(x, Wq, Wk, Wv):
    from concourse.bass_utils import run_bass_kernel_spmd

    nc = _get_compiled()
    in_maps = _host_inputs(x, Wq, Wk, Wv)
    res = run_bass_kernel_spmd(nc, in_maps, core_ids=list(range(N_CORES)))

    out_full = np.empty((B, T, D), dtype=np.float32)
    for core in range(N_CORES):
        b, p = core // 2, core % 2
        acc = res.results[core]["out"]          # (65, 2048) f32
        out_full[b, p::2, :] = (acc[:D] / acc[D:D + 1]).T
    return out_full
